# revision 31
# baseline (speedup 1.0000x reference)
"""Trainium2 Bass kernel for GQA attention (B=2,T=2048,D=2048,Hq=16,Hkv=4,Dh=128)
with RMSNorm + YaRN RoPE on q/k, causal softmax, out projection.

Sharding (tensor-parallel over heads, uniform SPMD across 8 cores):
  core c -> kv group g = c//2, query heads {g + 8*(c%2), g + 8*(c%2) + 4}.
  Each core computes the fused qkv projection for its 2 query heads + one
  128-col half of its kv group (k on even cores, v on odd), a pairwise
  AllGather shares k/v within the pair, each core runs full causal attention
  for its 2 heads, a per-(batch,head) AllToAll redistributes attention output
  from head-sharded to token-sharded, and every core runs the out-projection
  against full Wo for its 1/8 token slice. Host concatenates token slices.

Engine plan (vs the phase-sequential baseline):
  - ACT runs ONLY Ln/Exp (one table, zero ACT_TABLE_LOAD thrash):
    rv = rsqrt(ms) computed as exp(-0.5*ln(ms)); k-side rv is folded into the
    score-exp per-partition scale so k is roped unnormalized.
  - ATTN_SCALE and the rms weights are folded into the rope tables (bf16).
  - sum-of-squares via scalar_tensor_tensor+accum on GpSimd (Pool).
  - causal diag masking via gpsimd.affine_select zeroing AFTER exp.
  - kv pair-exchange split into 4-tb groups; emission interleaves proj(1)
    into attn(0,*) and oproj into attn(1,*) so the PE never idles.
  - Wo loaded once (4 persistent 512-col chunks) and shared by both batches.
"""

import math

import numpy as np
import ml_dtypes

import concourse.bass as bass
import concourse.tile as tile
from concourse import bacc, mybir
from concourse.bass_utils import run_bass_kernel_spmd
from concourse.masks import make_identity

# ---- problem constants --------------------------------------------------
B = 2
T = 2048
D_MODEL = 2048
D_HEAD = 128
N_Q, N_KV = 16, 4
ROPE_BASE = 10000.0
YARN_SCALE = 4.0
ORIG_MAX_LEN = 4096
BETA_FAST, BETA_SLOW = 32.0, 1.0
RMS_EPS = 1.1920929e-07
MSCALE = 0.1 * math.log(YARN_SCALE) + 1.0
ATTN_SCALE = 1.0 / (MSCALE * math.sqrt(D_HEAD))

N_CORES = 8
KT = D_MODEL // 128          # 16 contraction tiles
QC = 4                       # query blocks per attention chunk
SCHEDULE = "ilv"             # "seq" (phase-sequential) or "ilv" (interleaved)
EXP_SCALE_AP = False         # fold rv_k into exp scale (AP) vs TS on kn
AFFINE_MASK = False          # causal mask via gpsimd.affine_select post-exp
PREFIX = 4                   # debug: emit only the first N phases (4 = all)
FIRE_KV = True              # debug: emit kv collectives
TB = T // 128                # 16 token blocks
BPS = TB // N_CORES          # 2 token blocks per core after A2A
BF16 = mybir.dt.bfloat16
F32 = mybir.dt.float32
Alu = mybir.AluOpType
Act = mybir.ActivationFunctionType


def _core_heads(c):
    g = c // 2
    ha = g + 8 * (c % 2)
    return g, (ha, ha + 4)


# o-proj: head id (Wo row block) owning A2A-out slot (r, hl); same all cores
AD_OF = [h for r in range(N_CORES) for h in _core_heads(r)[1]]


# ---- bass graph ---------------------------------------------------------

def build_nc():
    nc = bacc.Bacc("TRN2", target_bir_lowering=False, debug=False,
                   num_devices=N_CORES)

    xT = nc.dram_tensor("xT", [B, D_MODEL, T], BF16, kind="ExternalInput")
    wc = nc.dram_tensor("wc", [KT, 128, 384], BF16, kind="ExternalInput")
    wo = nc.dram_tensor("wo", [KT, 128, D_MODEL], BF16, kind="ExternalInput")
    c2q = nc.dram_tensor("c2q", [T, 256], BF16, kind="ExternalInput")
    s2q = nc.dram_tensor("s2q", [T, 256], BF16, kind="ExternalInput")
    ck = nc.dram_tensor("ck", [T, 128], BF16, kind="ExternalInput")
    sk = nc.dram_tensor("sk", [T, 128], BF16, kind="ExternalInput")
    out = nc.dram_tensor("out", [B, BPS, 128, D_MODEL], BF16,
                         kind="ExternalOutput")

    with tile.TileContext(nc) as tc:
        with (
            tc.tile_pool(name="const", bufs=1) as constp,
            tc.tile_pool(name="xin", bufs=4) as xin,
            tc.tile_pool(name="persist", bufs=1) as persist,
            tc.tile_pool(name="work", bufs=6) as work,
            tc.tile_pool(name="ps512", bufs=3, space="PSUM") as ps512,
            tc.tile_pool(name="ps_att", bufs=4, space="PSUM") as ps_att,
            tc.tile_pool(name="ps_tr", bufs=1, space="PSUM") as ps_tr,
            tc.tile_pool(name="dram", bufs=1, space="DRAM") as dram,
        ):
            # ---- prologue DMAs (first matmul needs only wc0 + xt(0,0)) ---
            wc0_t = constp.tile([128, 384], BF16, tag="wc0", name="wc0")
            nc.sync.dma_start(out=wc0_t, in_=wc.ap()[0])

            xt_t = {}

            def load_xt(b, tb):
                t_ = xin.tile([128, KT, 128], BF16, tag="xt", name="xt")
                nc.sync.dma_start(
                    out=t_,
                    in_=xT.ap()[b].rearrange("(k p) t -> p k t", p=128)
                        [:, :, tb * 128:(tb + 1) * 128])
                xt_t[(b, tb)] = t_

            load_xt(0, 0)
            load_xt(0, 1)

            wcr_t = constp.tile([128, KT - 1, 384], BF16, tag="wcr",
                                name="wcr")
            nc.sync.dma_start(
                out=wcr_t, in_=wc.ap()[1:KT].rearrange("k p n -> p k n"))

            def wc_kt(kt):
                return wc0_t if kt == 0 else wcr_t[:, kt - 1, :]

            ident = constp.tile([128, 128], BF16, tag="ident", name="ident")
            make_identity(nc, ident)

            c2q_t = constp.tile([128, TB, 2, 128], BF16, tag="c2q",
                                name="c2q")
            nc.gpsimd.dma_start(
                out=c2q_t,
                in_=c2q.ap().rearrange("(tb p) (h d) -> p tb h d", p=128,
                                       h=2))
            s2q_t = constp.tile([128, TB, 2, 128], BF16, tag="s2q",
                                name="s2q")
            nc.gpsimd.dma_start(
                out=s2q_t,
                in_=s2q.ap().rearrange("(tb p) (h d) -> p tb h d", p=128,
                                       h=2))
            ck_t = constp.tile([128, TB, 128], BF16, tag="ck", name="ck")
            nc.gpsimd.dma_start(
                out=ck_t, in_=ck.ap().rearrange("(tb p) d -> p tb d", p=128))
            sk_t = constp.tile([128, TB, 128], BF16, tag="sk", name="sk")
            nc.gpsimd.dma_start(
                out=sk_t, in_=sk.ap().rearrange("(tb p) d -> p tb d", p=128))

            eps_sb = constp.tile([128, 1], F32, tag="eps", name="eps")
            nc.vector.memset(eps_sb, RMS_EPS)

            mask_sb = None
            if not AFFINE_MASK:
                # causal mask for the diagonal block: 0 keep / -1e30 drop
                # out[k, q]: keep iff k <= q
                mask_sb = constp.tile([128, 128], F32, tag="mask",
                                      name="mask")
                nc.gpsimd.memset(mask_sb, 0.0)
                nc.gpsimd.affine_select(
                    out=mask_sb, in_=mask_sb, compare_op=Alu.is_ge,
                    fill=-1e30, base=0, pattern=[[1, 128]],
                    channel_multiplier=-1)

            # ---- dram bounce buffers -------------------------------------
            kv_in, kv_out = {}, {}
            for b in range(B):
                kv_in[b] = dram.tile([TB, 128, 128], BF16,
                                     tag=f"kvin{b}", name=f"kvin{b}")
                kv_out[b] = dram.tile([TB // 4, 2, 4, 128, 128], BF16,
                                      tag=f"kvout{b}", name=f"kvout{b}")
            a2a_in, a2a_out = {}, {}
            for b in range(B):
                for hl in range(2):
                    for be in range(BPS):
                        a2a_in[(b, hl, be)] = dram.tile(
                            [N_CORES, 128, 128], BF16,
                            tag=f"a2ain{b}_{hl}_{be}",
                            name=f"a2ain{b}_{hl}_{be}")
                        a2a_out[(b, hl, be)] = dram.tile(
                            [N_CORES, 128, 128], BF16,
                            tag=f"a2aout{b}_{hl}_{be}",
                            name=f"a2aout{b}_{hl}_{be}")

            # ---- persistent state ----------------------------------------
            qTbig, kTt, vA, rvk = {}, {}, {}, {}
            for b in range(B):
                for hl in range(2):
                    qTbig[(b, hl)] = persist.tile(
                        [128, TB * 128], BF16, tag=f"qTb{b}_{hl}",
                        name=f"qTb{b}_{hl}")
            ws_t = {}
            for c in range(4):
                ws_t[c] = persist.tile([128, KT, 512], BF16, tag=f"ws{c}",
                                       name=f"ws{c}")

            def load_ws(c):
                nc.sync.dma_start(
                    out=ws_t[c],
                    in_=wo.ap().rearrange("k p n -> p k n")
                        [:, :, c * 512:(c + 1) * 512])

            # ---- phase pieces --------------------------------------------

            def proj_block(b, tb):
                """q+kv projection, rms (Pool sumsq + ACT ln/exp), rope,
                transpose for one 128-token block."""
                xt = xt_t.pop((b, tb))
                ps = ps512.tile([128, 512], F32, tag="p512", name="p512")
                for kt in range(KT):
                    nc.tensor.matmul(ps[:, 0:384], xt[:, kt, :], wc_kt(kt),
                                     start=(kt == 0), stop=(kt == KT - 1))
                # raw kv half out to the pair-exchange buffer (ACT copy;
                # Copy lives in every activation table so no table switch)
                kvr = work.tile([128, 128], BF16, tag="kvr", name="kvr")
                nc.scalar.activation(kvr, ps[:, 256:384], Act.Copy)
                nc.sync.dma_start(out=kv_in[b][tb], in_=kvr)
                # raw q to SBUF bf16 (releases the psum bank early)
                qc = work.tile([128, 2, 128], BF16, tag="qc", name="qc")
                nc.vector.tensor_copy(qc, ps[:, 0:256])
                # rms sumsq for the 2 q heads (square + free-dim reduce)
                ssq = work.tile([128, 2], F32, tag="ssq", name="ssq")
                for h in range(2):
                    scr = work.tile([128, 128], BF16, tag="sqscr",
                                    name="sqscr", bufs=3)
                    nc.vector.tensor_tensor(scr, qc[:, h, :], qc[:, h, :],
                                            Alu.mult)
                    nc.vector.tensor_reduce(ssq[:, h:h + 1], scr,
                                            mybir.AxisListType.X, Alu.add)
                lnv = work.tile([128, 2], F32, tag="lnv", name="lnv")
                nc.scalar.activation(lnv, ssq, Act.Ln, bias=eps_sb,
                                     scale=1.0 / 128.0)
                rvq = work.tile([128, 2], F32, tag="rvq", name="rvq")
                nc.scalar.activation(rvq, lnv, Act.Exp, scale=-0.5)
                # normalize to bf16 (ATTN_SCALE lives in the tables)
                qs = work.tile([128, 2, 128], BF16, tag="qs", name="qs")
                for h in range(2):
                    nc.vector.tensor_scalar_mul(qs[:, h, :], qc[:, h, :],
                                                rvq[:, h:h + 1])
                # rope, both heads batched (tables duplicated per head)
                t1 = work.tile([128, 2, 128], BF16, tag="t1", name="t1")
                nc.vector.tensor_tensor(t1, qs, c2q_t[:, tb], Alu.mult)
                t2 = work.tile([128, 2, 128], BF16, tag="t2", name="t2")
                nc.vector.tensor_tensor(t2[:, :, 0:64], qs[:, :, 64:128],
                                        s2q_t[:, tb, :, 0:64], Alu.mult)
                nc.vector.tensor_tensor(t2[:, :, 64:128], qs[:, :, 0:64],
                                        s2q_t[:, tb, :, 64:128], Alu.mult)
                qn = work.tile([128, 2, 128], BF16, tag="qn", name="qn")
                nc.vector.tensor_add(qn, t1, t2)
                for hl in range(2):
                    tr = ps_tr.tile([128, 128], BF16, tag="tr", name="tr")
                    nc.tensor.transpose(tr, qn[:, hl, :], ident)
                    nc.vector.tensor_copy(
                        qTbig[(b, hl)][:, tb * 128:(tb + 1) * 128], tr)

            def fire_kv(b, g4):
                """pair AllGather of raw k/v halves for tb in [4g4, 4g4+4)."""
                s = slice(g4 * 4, g4 * 4 + 4)
                nc.gpsimd.collective_compute(
                    "AllGather", Alu.bypass,
                    replica_groups=[[0, 1], [2, 3], [4, 5], [6, 7]],
                    ins=[kv_in[b][s].opt()],
                    outs=[kv_out[b][g4].opt()])

            def postag_group(b, g):
                """post-exchange k rope (unnormalized) + rv_k + v staging
                for tb in {2g, 2g+1}."""
                kraws = []
                kssq = work.tile([128, 2], F32, tag="kssq", name="kssq")
                for i in range(2):
                    tb = 2 * g + i
                    kraw = work.tile([128, 128], BF16, tag="kraw",
                                     name="kraw", bufs=4)
                    nc.gpsimd.dma_start(out=kraw,
                                        in_=kv_out[b][tb // 4, 0, tb % 4])
                    kraws.append(kraw)
                    va = persist.tile([128, 129], BF16, tag=f"vA_{b}_{tb}",
                                      name=f"vA_{b}_{tb}")
                    vA[(b, tb)] = va
                    nc.gpsimd.dma_start(out=va[:, 0:128],
                                        in_=kv_out[b][tb // 4, 1, tb % 4])
                    nc.gpsimd.memset(va[:, 128:129], 1.0)
                    scr = work.tile([128, 128], BF16, tag="sqscr",
                                    name="sqscr", bufs=3)
                    nc.vector.tensor_tensor(scr, kraw, kraw, Alu.mult)
                    nc.vector.tensor_reduce(kssq[:, i:i + 1], scr,
                                            mybir.AxisListType.X, Alu.add)
                klnv = work.tile([128, 2], F32, tag="klnv", name="klnv")
                nc.scalar.activation(klnv, kssq, Act.Ln, bias=eps_sb,
                                     scale=1.0 / 128.0)
                rv = persist.tile([128, 2], F32, tag=f"rvk_{b}_{g}",
                                  name=f"rvk_{b}_{g}")
                rvk[(b, g)] = rv
                nc.scalar.activation(rv, klnv, Act.Exp, scale=-0.5)
                for i in range(2):
                    tb = 2 * g + i
                    kraw = kraws[i]
                    # k rope on Pool (all-SBUF bf16), unnormalized
                    kt1 = work.tile([128, 128], BF16, tag="kt1", name="kt1")
                    nc.gpsimd.tensor_tensor(kt1, kraw, ck_t[:, tb], Alu.mult)
                    kt2 = work.tile([128, 128], BF16, tag="kt2", name="kt2")
                    nc.gpsimd.tensor_tensor(kt2[:, 0:64], kraw[:, 64:128],
                                            sk_t[:, tb, 0:64], Alu.mult)
                    nc.gpsimd.tensor_tensor(kt2[:, 64:128], kraw[:, 0:64],
                                            sk_t[:, tb, 64:128], Alu.mult)
                    kn = work.tile([128, 128], BF16, tag="kn", name="kn")
                    nc.gpsimd.tensor_add(kn, kt1, kt2)
                    if not EXP_SCALE_AP:
                        knn = work.tile([128, 128], BF16, tag="knn",
                                        name="knn")
                        nc.vector.tensor_scalar_mul(knn, kn,
                                                    rv[:, i:i + 1])
                        kn = knn
                    tr = ps_tr.tile([128, 128], BF16, tag="tr", name="tr")
                    nc.tensor.transpose(tr, kn, ident)
                    dst = persist.tile([128, 128], BF16, tag=f"kT_{b}_{tb}",
                                       name=f"kT_{b}_{tb}")
                    kTt[(b, tb)] = dst
                    nc.vector.tensor_copy(dst, tr)

            # ---- attention -----------------------------------------------
            attn_state = {}

            def attn_chunk(b, hl, j):
                """one chunk (4 q-blocks) of causal attention for head-half
                hl of batch b; software-pipelined scores/exp/PV; tails +
                A2A fires at be boundaries."""
                qTb = qTbig[(b, hl)]
                st = attn_state.setdefault((b, hl), {"deferred": None})
                q0 = j * QC
                pas = []

                def score_row(kb):
                    diag = kb >= q0
                    w = (q0 + QC - kb) if diag else QC
                    cols = slice((kb if diag else q0) * 128, (q0 + QC) * 128)
                    ss = ps512.tile([128, 512], F32, tag="p512", name="p512")
                    nc.tensor.matmul(ss[:, 0:w * 128], kTt[(b, kb)],
                                     qTb[:, cols], start=True, stop=True)
                    if diag and not AFFINE_MASK:
                        nc.vector.tensor_add(ss[:, 0:128], ss[:, 0:128],
                                             mask_sb)
                    ptw = work.tile([128, 512], BF16, tag="ptw", name="ptw",
                                    bufs=8)
                    if EXP_SCALE_AP:
                        nc.scalar.activation(
                            ptw[:, 0:w * 128], ss[:, 0:w * 128], Act.Exp,
                            scale=rvk[(b, kb // 2)][:, kb % 2:kb % 2 + 1])
                    else:
                        nc.scalar.activation(ptw[:, 0:w * 128],
                                             ss[:, 0:w * 128], Act.Exp)
                    if diag and AFFINE_MASK:
                        # zero the strictly-upper triangle of the diag block
                        nc.gpsimd.affine_select(
                            out=ptw[:, 0:128], in_=ptw[:, 0:128],
                            pattern=[[1, 128]], compare_op=Alu.is_ge,
                            fill=0.0, base=0, channel_multiplier=-1)
                    return ptw

                def pv_row(kb, ptw):
                    if not pas:
                        pas.extend(ps_att.tile([128, 129], F32, tag="pa",
                                               name="pa")
                                   for _ in range(QC))
                    diag = kb >= q0
                    for qq in range(kb - q0 if diag else 0, QC):
                        off = (qq - (kb - q0)) if diag else qq
                        nc.tensor.matmul(
                            pas[qq], ptw[:, off * 128:(off + 1) * 128],
                            vA[(b, kb)],
                            start=(kb == 0), stop=(q0 + qq == kb))

                def tail(qq):
                    qb = q0 + qq
                    pa = pas[qq]
                    rv = work.tile([128, 1], F32, tag="rsum", name="rsum")
                    nc.vector.reciprocal(rv, pa[:, 128:129])
                    an = work.tile([128, 128], BF16, tag="attn_n",
                                   name="attn_n")
                    nc.vector.tensor_scalar_mul(an, pa[:, 0:128], rv)
                    tr = ps_tr.tile([128, 128], BF16, tag="tr", name="tr")
                    nc.tensor.transpose(tr, an, ident)
                    at = work.tile([128, 128], BF16, tag="attnT",
                                   name="attnT")
                    nc.vector.tensor_copy(at, tr)
                    nc.gpsimd.dma_start(
                        out=a2a_in[(b, hl, qb // N_CORES)][qb % N_CORES],
                        in_=at)

                def fire():
                    if (q0 + QC) % N_CORES == 0:
                        be = (q0 + QC) // N_CORES - 1
                        nc.gpsimd.collective_compute(
                            "AllToAll", Alu.bypass,
                            replica_groups=[list(range(N_CORES))],
                            ins=[a2a_in[(b, hl, be)].opt()],
                            outs=[a2a_out[(b, hl, be)].opt()])

                rows = list(range(q0 + QC))
                prev = None
                for i, kb in enumerate(rows):
                    cur = score_row(kb)
                    if i == 0 and st["deferred"] is not None:
                        tl, fr = st["deferred"]
                        st["deferred"] = None
                        tl(QC - 1)
                        fr()
                    if prev is not None:
                        pv_row(rows[i - 1], prev)
                    if kb > q0:
                        tail(kb - q0 - 1)
                    prev = cur
                pv_row(rows[-1], prev)
                st["deferred"] = (tail, fire)

            def attn_finish(b, hl):
                tl, fr = attn_state[(b, hl)]["deferred"]
                attn_state[(b, hl)]["deferred"] = None
                tl(QC - 1)
                fr()

            # ---- out projection ------------------------------------------
            gs_t = {}

            def load_gs(b, blk):
                g = persist.tile([128, 16, 128], BF16, tag=f"aG_{b}_{blk}",
                                 name=f"aG_{b}_{blk}")
                gs_t[(b, blk)] = g
                gr = g.rearrange("p (r hl) t -> p r hl t", hl=2)
                for hl in range(2):
                    nc.sync.dma_start(
                        out=gr[:, :, hl, :],
                        in_=a2a_out[(b, hl, blk)][:, :, :]
                            .rearrange("r p t -> p r t"))

            def oproj_piece(b, blk, chunk):
                g = gs_t[(b, blk)]
                po = ps512.tile([128, 512], F32, tag="p512", name="p512")
                for idx in range(16):
                    nc.tensor.matmul(po, g[:, idx, :],
                                     ws_t[chunk][:, AD_OF[idx], :],
                                     start=(idx == 0), stop=(idx == 15))
                os_ = work.tile([128, 512], BF16, tag="os", name="os",
                                bufs=3)
                nc.scalar.activation(os_, po, Act.Copy)
                nc.scalar.dma_start(
                    out=out.ap()[b, blk, :, chunk * 512:(chunk + 1) * 512],
                    in_=os_)

            # ---- emission schedule ---------------------------------------
            def proj1_pair(p):
                for i in range(2):
                    tb = 2 * p + i
                    if tb + 2 < TB:
                        load_xt(1, tb + 2)
                    proj_block(1, tb)
                if p % 2 == 1:
                    fire_kv(1, p // 2)

            def dump(sbuf_tile, col=0):
                """debug: land something in `out` so the NEFF has output."""
                nc.scalar.dma_start(
                    out=out.ap()[0, 0, :, col * 128:(col + 1) * 128],
                    in_=sbuf_tile)

            if SCHEDULE == "seq":
                for tb in range(TB):
                    if tb + 2 < TB:
                        load_xt(0, tb + 2)
                    elif tb + 2 < TB + 3:
                        load_xt(1, tb + 2 - TB)
                    proj_block(0, tb)
                    if FIRE_KV and tb % 4 == 3:
                        fire_kv(0, tb // 4)
                if PREFIX == 1:
                    dump(qTbig[(0, 0)][:, 0:128])
                if PREFIX >= 2:
                    for p in range(8):
                        proj1_pair(p)
                    for g in range(8):
                        postag_group(0, g)
                    if PREFIX == 2:
                        dump(kTt[(0, 0)])
                if PREFIX >= 3:
                    for hl in range(2):
                        for j in range(4):
                            attn_chunk(0, hl, j)
                        attn_finish(0, hl)
                    for g in range(8):
                        postag_group(1, g)
                    if PREFIX == 3:
                        dump(kTt[(1, 0)])
                if PREFIX >= 4:
                    for c in range(4):
                        load_ws(c)
                    for hl in range(2):
                        for j in range(4):
                            attn_chunk(1, hl, j)
                        attn_finish(1, hl)
                    for b in range(B):
                        for blk in range(BPS):
                            load_gs(b, blk)
                            for c in range(4):
                                oproj_piece(b, blk, c)
            else:
                # proj(0) with per-4tb kv fires; post_ag(0) interleaved
                for tb in range(TB):
                    if tb + 2 < TB:
                        load_xt(0, tb + 2)
                    elif tb + 2 < TB + 3:
                        load_xt(1, tb + 2 - TB)
                    proj_block(0, tb)
                    if tb % 4 == 3:
                        fire_kv(0, tb // 4)
                    if tb == 9:
                        postag_group(0, 0)
                    if tb == 11:
                        postag_group(0, 1)
                    if tb == 13:
                        postag_group(0, 2)
                    if tb == 15:
                        postag_group(0, 3)
                postag_group(0, 4)
                postag_group(0, 5)
                attn_chunk(0, 0, 0)
                postag_group(0, 6)
                postag_group(0, 7)
                attn_chunk(0, 0, 1)
                proj1_pair(0)
                attn_chunk(0, 0, 2)
                proj1_pair(1)
                attn_chunk(0, 0, 3)
                proj1_pair(2)
                attn_finish(0, 0)
                attn_chunk(0, 1, 0)
                proj1_pair(3)
                attn_chunk(0, 1, 1)
                proj1_pair(4)
                load_ws(0)
                attn_chunk(0, 1, 2)
                proj1_pair(5)
                load_ws(1)
                attn_chunk(0, 1, 3)
                proj1_pair(6)
                attn_finish(0, 1)
                proj1_pair(7)
                load_ws(2)
                load_ws(3)
                postag_group(1, 0)
                postag_group(1, 1)
                attn_chunk(1, 0, 0)
                postag_group(1, 2)
                postag_group(1, 3)
                attn_chunk(1, 0, 1)
                postag_group(1, 4)
                postag_group(1, 5)
                load_gs(0, 0)
                oproj_piece(0, 0, 0)
                oproj_piece(0, 0, 1)
                attn_chunk(1, 0, 2)
                postag_group(1, 6)
                postag_group(1, 7)
                oproj_piece(0, 0, 2)
                oproj_piece(0, 0, 3)
                attn_chunk(1, 0, 3)
                attn_finish(1, 0)
                load_gs(0, 1)
                oproj_piece(0, 1, 0)
                oproj_piece(0, 1, 1)
                attn_chunk(1, 1, 0)
                oproj_piece(0, 1, 2)
                oproj_piece(0, 1, 3)
                attn_chunk(1, 1, 1)
                attn_chunk(1, 1, 2)
                load_gs(1, 0)
                oproj_piece(1, 0, 0)
                oproj_piece(1, 0, 1)
                attn_chunk(1, 1, 3)
                attn_finish(1, 1)
                oproj_piece(1, 0, 2)
                oproj_piece(1, 0, 3)
                load_gs(1, 1)
                for c in range(4):
                    oproj_piece(1, 1, c)
    nc.compile()
    return nc


# ---- host side ----------------------------------------------------------

def _yarn_tables(t_tokens):
    inv = 1.0 / ROPE_BASE ** (np.arange(0, D_HEAD, 2, dtype=np.float32) / D_HEAD)
    wavelengths = 2.0 * math.pi / inv
    low_wl = ORIG_MAX_LEN / BETA_SLOW
    high_wl = ORIG_MAX_LEN / BETA_FAST
    gamma = np.clip((low_wl - wavelengths) / (low_wl - high_wl), 0.0, 1.0)
    inv_freq = (gamma * inv + (1.0 - gamma) * inv / YARN_SCALE).astype(np.float32)
    t = np.arange(t_tokens, dtype=np.float32)
    freqs = np.outer(t, inv_freq)                      # (T, 64)
    emb = np.concatenate([freqs, freqs], axis=-1)      # (T, 128)
    return np.cos(emb).astype(np.float32), np.sin(emb).astype(np.float32)


def _host_prep(x, Wq, Wkv, Wo, q_norm_w, k_norm_w):
    bf = ml_dtypes.bfloat16
    xT = np.ascontiguousarray(x.transpose(0, 2, 1)).astype(bf)   # (B, D, T)
    cos, sin = _yarn_tables(T)
    sinF = sin.copy()
    sinF[:, :64] *= -1.0
    # rms weight applies to x before rope; the sin term reads the *rotated*
    # input, so its weight index is the input position (rolled by 64).
    wq_roll = np.concatenate([q_norm_w[64:], q_norm_w[:64]])
    wk_roll = np.concatenate([k_norm_w[64:], k_norm_w[:64]])
    cq = (cos * q_norm_w[None, :] * ATTN_SCALE).astype(bf)
    sq = (sinF * wq_roll[None, :] * ATTN_SCALE).astype(bf)
    c2q = np.ascontiguousarray(np.concatenate([cq, cq], axis=1))  # (T, 256)
    s2q = np.ascontiguousarray(np.concatenate([sq, sq], axis=1))
    ckt = np.ascontiguousarray((cos * k_norm_w[None, :]).astype(bf))
    skt = np.ascontiguousarray((sinF * wk_roll[None, :]).astype(bf))
    Wk, Wv = Wkv[:, :N_KV * D_HEAD], Wkv[:, N_KV * D_HEAD:]
    wo_t = np.ascontiguousarray(Wo.astype(bf).reshape(KT, 128, D_MODEL))
    in_maps = []
    for c in range(N_CORES):
        g, (ha, hb) = _core_heads(c)
        kv_half = (Wk if c % 2 == 0 else Wv)[:, g * 128:(g + 1) * 128]
        wcols = np.concatenate([
            Wq[:, ha * 128:(ha + 1) * 128], Wq[:, hb * 128:(hb + 1) * 128],
            kv_half,
        ], axis=1).astype(bf)                               # (D, 384)
        in_maps.append({
            "xT": xT, "wc": np.ascontiguousarray(wcols.reshape(KT, 128, 384)),
            "wo": wo_t,
            "c2q": c2q, "s2q": s2q, "ck": ckt, "sk": skt,
        })
    return in_maps


def _assemble(results):
    out = np.empty((B, T, D_MODEL), dtype=np.float32)
    for c in range(N_CORES):
        oc = results[c]["out"]              # (B, BPS, 128, D) bf16
        for b in range(B):
            for blk in range(BPS):
                t0 = (c + blk * N_CORES) * 128
                out[b, t0:t0 + 128, :] = oc[b, blk].astype(np.float32)
    return out


_NC_CACHE = {}


def kernel(x, Wq, Wkv, Wo, q_norm_w, k_norm_w):
    x = np.asarray(x, dtype=np.float32)
    Wq = np.asarray(Wq, dtype=np.float32)
    Wkv = np.asarray(Wkv, dtype=np.float32)
    Wo = np.asarray(Wo, dtype=np.float32)
    q_norm_w = np.asarray(q_norm_w, dtype=np.float32)
    k_norm_w = np.asarray(k_norm_w, dtype=np.float32)

    if "nc" not in _NC_CACHE:
        _NC_CACHE["nc"] = build_nc()
    nc = _NC_CACHE["nc"]
    in_maps = _host_prep(x, Wq, Wkv, Wo, q_norm_w, k_norm_w)
    res = run_bass_kernel_spmd(nc, in_maps, core_ids=list(range(N_CORES)))
    return _assemble(res.results)


if __name__ == "__main__":
    rng = np.random.default_rng(0)
    x = rng.standard_normal((B, T, D_MODEL), dtype=np.float32)
    Wq = rng.standard_normal((D_MODEL, N_Q * D_HEAD), dtype=np.float32) * 0.02
    Wkv = rng.standard_normal((D_MODEL, 2 * N_KV * D_HEAD), dtype=np.float32) * 0.02
    Wo = rng.standard_normal((N_Q * D_HEAD, D_MODEL), dtype=np.float32) * 0.02
    w1 = np.ones(D_HEAD, dtype=np.float32)
    o = kernel(x, Wq, Wkv, Wo, w1, w1)
    print(o.shape, o.dtype, float(np.abs(o).mean()))


# revision 33
# speedup vs baseline: 1.0086x; 1.0086x over previous
"""Trainium2 Bass kernel for GQA attention (B=2,T=2048,D=2048,Hq=16,Hkv=4,Dh=128)
with RMSNorm + YaRN RoPE on q/k, causal softmax, out projection.

Sharding (tensor-parallel over heads, uniform SPMD across 8 cores):
  core c -> kv group g = c//2, query heads {g + 8*(c%2), g + 8*(c%2) + 4}.
  Each core computes the fused qkv projection for its 2 query heads + one
  128-col half of its kv group (k on even cores, v on odd), a pairwise
  AllGather shares k/v within the pair, each core runs full causal attention
  for its 2 heads, a per-(batch,head) AllToAll redistributes attention output
  from head-sharded to token-sharded, and every core runs the out-projection
  against full Wo for its 1/8 token slice. Host concatenates token slices.

Engine plan (vs the phase-sequential baseline):
  - ACT runs ONLY Ln/Exp (one table, zero ACT_TABLE_LOAD thrash):
    rv = rsqrt(ms) computed as exp(-0.5*ln(ms)); k-side rv is folded into the
    score-exp per-partition scale so k is roped unnormalized.
  - ATTN_SCALE and the rms weights are folded into the rope tables (bf16).
  - sum-of-squares via scalar_tensor_tensor+accum on GpSimd (Pool).
  - causal diag masking via gpsimd.affine_select zeroing AFTER exp.
  - kv pair-exchange split into 4-tb groups; emission interleaves proj(1)
    into attn(0,*) and oproj into attn(1,*) so the PE never idles.
  - Wo loaded once (4 persistent 512-col chunks) and shared by both batches.
"""

import math

import numpy as np
import ml_dtypes

import concourse.bass as bass
import concourse.tile as tile
from concourse import bacc, mybir
from concourse.bass_utils import run_bass_kernel_spmd
from concourse.masks import make_identity

# ---- problem constants --------------------------------------------------
B = 2
T = 2048
D_MODEL = 2048
D_HEAD = 128
N_Q, N_KV = 16, 4
ROPE_BASE = 10000.0
YARN_SCALE = 4.0
ORIG_MAX_LEN = 4096
BETA_FAST, BETA_SLOW = 32.0, 1.0
RMS_EPS = 1.1920929e-07
MSCALE = 0.1 * math.log(YARN_SCALE) + 1.0
ATTN_SCALE = 1.0 / (MSCALE * math.sqrt(D_HEAD))

N_CORES = 8
KT = D_MODEL // 128          # 16 contraction tiles
QC = 4                       # query blocks per attention chunk
SCHEDULE = "ilv"             # "seq" (phase-sequential) or "ilv" (interleaved)
EXP_SCALE_AP = True         # fold rv_k into exp scale (AP) vs TS on kn
AFFINE_MASK = False          # causal mask via gpsimd.affine_select post-exp
PREFIX = 4                   # debug: emit only the first N phases (4 = all)
FIRE_KV = True              # debug: emit kv collectives
TB = T // 128                # 16 token blocks
BPS = TB // N_CORES          # 2 token blocks per core after A2A
BF16 = mybir.dt.bfloat16
F32 = mybir.dt.float32
Alu = mybir.AluOpType
Act = mybir.ActivationFunctionType


def _core_heads(c):
    g = c // 2
    ha = g + 8 * (c % 2)
    return g, (ha, ha + 4)


# o-proj: head id (Wo row block) owning A2A-out slot (r, hl); same all cores
AD_OF = [h for r in range(N_CORES) for h in _core_heads(r)[1]]


# ---- bass graph ---------------------------------------------------------

def build_nc():
    nc = bacc.Bacc("TRN2", target_bir_lowering=False, debug=False,
                   num_devices=N_CORES)

    xT = nc.dram_tensor("xT", [B, D_MODEL, T], BF16, kind="ExternalInput")
    wc = nc.dram_tensor("wc", [KT, 128, 384], BF16, kind="ExternalInput")
    wo = nc.dram_tensor("wo", [KT, 128, D_MODEL], BF16, kind="ExternalInput")
    c2q = nc.dram_tensor("c2q", [T, 256], BF16, kind="ExternalInput")
    s2q = nc.dram_tensor("s2q", [T, 256], BF16, kind="ExternalInput")
    ck = nc.dram_tensor("ck", [T, 128], BF16, kind="ExternalInput")
    sk = nc.dram_tensor("sk", [T, 128], BF16, kind="ExternalInput")
    out = nc.dram_tensor("out", [B, BPS, 128, D_MODEL], BF16,
                         kind="ExternalOutput")

    with tile.TileContext(nc) as tc:
        with (
            tc.tile_pool(name="const", bufs=1) as constp,
            tc.tile_pool(name="xin", bufs=4) as xin,
            tc.tile_pool(name="persist", bufs=1) as persist,
            tc.tile_pool(name="work", bufs=6) as work,
            tc.tile_pool(name="ps512", bufs=3, space="PSUM") as ps512,
            tc.tile_pool(name="ps_att", bufs=4, space="PSUM") as ps_att,
            tc.tile_pool(name="ps_tr", bufs=1, space="PSUM") as ps_tr,
            tc.tile_pool(name="dram", bufs=1, space="DRAM") as dram,
        ):
            # ---- prologue DMAs (first matmul needs only wc0 + xt(0,0)) ---
            wc0_t = constp.tile([128, 384], BF16, tag="wc0", name="wc0")
            nc.sync.dma_start(out=wc0_t, in_=wc.ap()[0])

            xt_t = {}

            def load_xt(b, tb):
                t_ = xin.tile([128, KT, 128], BF16, tag="xt", name="xt")
                nc.sync.dma_start(
                    out=t_,
                    in_=xT.ap()[b].rearrange("(k p) t -> p k t", p=128)
                        [:, :, tb * 128:(tb + 1) * 128])
                xt_t[(b, tb)] = t_

            load_xt(0, 0)
            load_xt(0, 1)

            wcr_t = constp.tile([128, KT - 1, 384], BF16, tag="wcr",
                                name="wcr")
            nc.sync.dma_start(
                out=wcr_t, in_=wc.ap()[1:KT].rearrange("k p n -> p k n"))

            def wc_kt(kt):
                return wc0_t if kt == 0 else wcr_t[:, kt - 1, :]

            ident = constp.tile([128, 128], BF16, tag="ident", name="ident")
            make_identity(nc, ident)

            c2q_t = constp.tile([128, TB, 2, 128], BF16, tag="c2q",
                                name="c2q")
            nc.gpsimd.dma_start(
                out=c2q_t,
                in_=c2q.ap().rearrange("(tb p) (h d) -> p tb h d", p=128,
                                       h=2))
            s2q_t = constp.tile([128, TB, 2, 128], BF16, tag="s2q",
                                name="s2q")
            nc.gpsimd.dma_start(
                out=s2q_t,
                in_=s2q.ap().rearrange("(tb p) (h d) -> p tb h d", p=128,
                                       h=2))
            ck_t = constp.tile([128, TB, 128], BF16, tag="ck", name="ck")
            nc.gpsimd.dma_start(
                out=ck_t, in_=ck.ap().rearrange("(tb p) d -> p tb d", p=128))
            sk_t = constp.tile([128, TB, 128], BF16, tag="sk", name="sk")
            nc.gpsimd.dma_start(
                out=sk_t, in_=sk.ap().rearrange("(tb p) d -> p tb d", p=128))

            eps_sb = constp.tile([128, 1], F32, tag="eps", name="eps")
            nc.vector.memset(eps_sb, RMS_EPS)

            mask_sb = None
            if not AFFINE_MASK:
                # causal mask for the diagonal block: 0 keep / -1e30 drop
                # out[k, q]: keep iff k <= q
                mask_sb = constp.tile([128, 128], F32, tag="mask",
                                      name="mask")
                nc.gpsimd.memset(mask_sb, 0.0)
                nc.gpsimd.affine_select(
                    out=mask_sb, in_=mask_sb, compare_op=Alu.is_ge,
                    fill=-1e30, base=0, pattern=[[1, 128]],
                    channel_multiplier=-1)

            # ---- dram bounce buffers -------------------------------------
            kv_in, kv_out = {}, {}
            for b in range(B):
                kv_in[b] = dram.tile([TB, 128, 128], BF16,
                                     tag=f"kvin{b}", name=f"kvin{b}")
                kv_out[b] = dram.tile([TB // 4, 2, 4, 128, 128], BF16,
                                      tag=f"kvout{b}", name=f"kvout{b}")
            a2a_in, a2a_out = {}, {}
            for b in range(B):
                for hl in range(2):
                    for be in range(BPS):
                        a2a_in[(b, hl, be)] = dram.tile(
                            [N_CORES, 128, 128], BF16,
                            tag=f"a2ain{b}_{hl}_{be}",
                            name=f"a2ain{b}_{hl}_{be}")
                        a2a_out[(b, hl, be)] = dram.tile(
                            [N_CORES, 128, 128], BF16,
                            tag=f"a2aout{b}_{hl}_{be}",
                            name=f"a2aout{b}_{hl}_{be}")

            # ---- persistent state ----------------------------------------
            qTbig, kTt, vA, rvk = {}, {}, {}, {}
            for b in range(B):
                for hl in range(2):
                    qTbig[(b, hl)] = persist.tile(
                        [128, TB * 128], BF16, tag=f"qTb{b}_{hl}",
                        name=f"qTb{b}_{hl}")
            ws_t = {}
            for c in range(4):
                ws_t[c] = persist.tile([128, KT, 512], BF16, tag=f"ws{c}",
                                       name=f"ws{c}")

            def load_ws(c):
                nc.sync.dma_start(
                    out=ws_t[c],
                    in_=wo.ap().rearrange("k p n -> p k n")
                        [:, :, c * 512:(c + 1) * 512])

            # ---- phase pieces --------------------------------------------

            def proj_block(b, tb):
                """q+kv projection, rms (Pool sumsq + ACT ln/exp), rope,
                transpose for one 128-token block."""
                xt = xt_t.pop((b, tb))
                ps = ps512.tile([128, 512], F32, tag="p512", name="p512")
                for kt in range(KT):
                    nc.tensor.matmul(ps[:, 0:384], xt[:, kt, :], wc_kt(kt),
                                     start=(kt == 0), stop=(kt == KT - 1))
                # raw kv half out to the pair-exchange buffer (ACT copy;
                # Copy lives in every activation table so no table switch)
                kvr = work.tile([128, 128], BF16, tag="kvr", name="kvr")
                nc.scalar.activation(kvr, ps[:, 256:384], Act.Copy)
                nc.sync.dma_start(out=kv_in[b][tb], in_=kvr)
                # raw q to SBUF bf16 (releases the psum bank early)
                qc = work.tile([128, 2, 128], BF16, tag="qc", name="qc")
                nc.vector.tensor_copy(qc, ps[:, 0:256])
                # rms sumsq for the 2 q heads (square + free-dim reduce)
                ssq = work.tile([128, 2], F32, tag="ssq", name="ssq")
                for h in range(2):
                    scr = work.tile([128, 128], BF16, tag="sqscr",
                                    name="sqscr", bufs=3)
                    nc.vector.tensor_tensor(scr, qc[:, h, :], qc[:, h, :],
                                            Alu.mult)
                    nc.vector.tensor_reduce(ssq[:, h:h + 1], scr,
                                            mybir.AxisListType.X, Alu.add)
                lnv = work.tile([128, 2], F32, tag="lnv", name="lnv")
                nc.scalar.activation(lnv, ssq, Act.Ln, bias=eps_sb,
                                     scale=1.0 / 128.0)
                rvq = work.tile([128, 2], F32, tag="rvq", name="rvq")
                nc.scalar.activation(rvq, lnv, Act.Exp, scale=-0.5)
                # normalize to bf16 (ATTN_SCALE lives in the tables)
                qs = work.tile([128, 2, 128], BF16, tag="qs", name="qs")
                for h in range(2):
                    nc.vector.tensor_scalar_mul(qs[:, h, :], qc[:, h, :],
                                                rvq[:, h:h + 1])
                # rope, both heads batched (tables duplicated per head)
                t1 = work.tile([128, 2, 128], BF16, tag="t1", name="t1")
                nc.vector.tensor_tensor(t1, qs, c2q_t[:, tb], Alu.mult)
                t2 = work.tile([128, 2, 128], BF16, tag="t2", name="t2")
                nc.vector.tensor_tensor(t2[:, :, 0:64], qs[:, :, 64:128],
                                        s2q_t[:, tb, :, 0:64], Alu.mult)
                nc.vector.tensor_tensor(t2[:, :, 64:128], qs[:, :, 0:64],
                                        s2q_t[:, tb, :, 64:128], Alu.mult)
                qn = work.tile([128, 2, 128], BF16, tag="qn", name="qn")
                nc.vector.tensor_add(qn, t1, t2)
                for hl in range(2):
                    tr = ps_tr.tile([128, 128], BF16, tag="tr", name="tr")
                    nc.tensor.transpose(tr, qn[:, hl, :], ident)
                    nc.vector.tensor_copy(
                        qTbig[(b, hl)][:, tb * 128:(tb + 1) * 128], tr)

            def fire_kv(b, g4):
                """pair AllGather of raw k/v halves for tb in [4g4, 4g4+4)."""
                s = slice(g4 * 4, g4 * 4 + 4)
                nc.gpsimd.collective_compute(
                    "AllGather", Alu.bypass,
                    replica_groups=[[0, 1], [2, 3], [4, 5], [6, 7]],
                    ins=[kv_in[b][s].opt()],
                    outs=[kv_out[b][g4].opt()])

            def postag_group(b, g):
                """post-exchange k rope (unnormalized) + rv_k + v staging
                for tb in {2g, 2g+1}."""
                kraws = []
                kssq = work.tile([128, 2], F32, tag="kssq", name="kssq")
                for i in range(2):
                    tb = 2 * g + i
                    kraw = work.tile([128, 128], BF16, tag="kraw",
                                     name="kraw", bufs=4)
                    nc.gpsimd.dma_start(out=kraw,
                                        in_=kv_out[b][tb // 4, 0, tb % 4])
                    kraws.append(kraw)
                    va = persist.tile([128, 129], BF16, tag=f"vA_{b}_{tb}",
                                      name=f"vA_{b}_{tb}")
                    vA[(b, tb)] = va
                    nc.gpsimd.dma_start(out=va[:, 0:128],
                                        in_=kv_out[b][tb // 4, 1, tb % 4])
                    nc.gpsimd.memset(va[:, 128:129], 1.0)
                    scr = work.tile([128, 128], BF16, tag="sqscr",
                                    name="sqscr", bufs=3)
                    nc.vector.tensor_tensor(scr, kraw, kraw, Alu.mult)
                    nc.vector.tensor_reduce(kssq[:, i:i + 1], scr,
                                            mybir.AxisListType.X, Alu.add)
                klnv = work.tile([128, 2], F32, tag="klnv", name="klnv")
                nc.scalar.activation(klnv, kssq, Act.Ln, bias=eps_sb,
                                     scale=1.0 / 128.0)
                rv = persist.tile([128, 2], F32, tag=f"rvk_{b}_{g}",
                                  name=f"rvk_{b}_{g}")
                rvk[(b, g)] = rv
                nc.scalar.activation(rv, klnv, Act.Exp, scale=-0.5)
                for i in range(2):
                    tb = 2 * g + i
                    kraw = kraws[i]
                    # k rope on Pool (all-SBUF bf16), unnormalized
                    kt1 = work.tile([128, 128], BF16, tag="kt1", name="kt1")
                    nc.vector.tensor_tensor(kt1, kraw, ck_t[:, tb], Alu.mult)
                    kt2 = work.tile([128, 128], BF16, tag="kt2", name="kt2")
                    nc.vector.tensor_tensor(kt2[:, 0:64], kraw[:, 64:128],
                                            sk_t[:, tb, 0:64], Alu.mult)
                    nc.vector.tensor_tensor(kt2[:, 64:128], kraw[:, 0:64],
                                            sk_t[:, tb, 64:128], Alu.mult)
                    kn = work.tile([128, 128], BF16, tag="kn", name="kn")
                    nc.vector.tensor_add(kn, kt1, kt2)
                    if not EXP_SCALE_AP:
                        knn = work.tile([128, 128], BF16, tag="knn",
                                        name="knn")
                        nc.vector.tensor_scalar_mul(knn, kn,
                                                    rv[:, i:i + 1])
                        kn = knn
                    tr = ps_tr.tile([128, 128], BF16, tag="tr", name="tr")
                    nc.tensor.transpose(tr, kn, ident)
                    dst = persist.tile([128, 128], BF16, tag=f"kT_{b}_{tb}",
                                       name=f"kT_{b}_{tb}")
                    kTt[(b, tb)] = dst
                    nc.vector.tensor_copy(dst, tr)

            # ---- attention -----------------------------------------------
            attn_state = {}

            def attn_chunk(b, hl, j):
                """one chunk (4 q-blocks) of causal attention for head-half
                hl of batch b; software-pipelined scores/exp/PV; tails +
                A2A fires at be boundaries."""
                qTb = qTbig[(b, hl)]
                st = attn_state.setdefault((b, hl), {"deferred": None})
                q0 = j * QC
                pas = []

                def score_row(kb):
                    diag = kb >= q0
                    w = (q0 + QC - kb) if diag else QC
                    cols = slice((kb if diag else q0) * 128, (q0 + QC) * 128)
                    ss = ps512.tile([128, 512], F32, tag="p512", name="p512")
                    nc.tensor.matmul(ss[:, 0:w * 128], kTt[(b, kb)],
                                     qTb[:, cols], start=True, stop=True)
                    if diag and not AFFINE_MASK:
                        nc.vector.tensor_add(ss[:, 0:128], ss[:, 0:128],
                                             mask_sb)
                    ptw = work.tile([128, 512], BF16, tag="ptw", name="ptw",
                                    bufs=8)
                    if EXP_SCALE_AP:
                        nc.scalar.activation(
                            ptw[:, 0:w * 128], ss[:, 0:w * 128], Act.Exp,
                            scale=rvk[(b, kb // 2)][:, kb % 2:kb % 2 + 1])
                    else:
                        nc.scalar.activation(ptw[:, 0:w * 128],
                                             ss[:, 0:w * 128], Act.Exp)
                    if diag and AFFINE_MASK:
                        # zero the strictly-upper triangle of the diag block
                        nc.gpsimd.affine_select(
                            out=ptw[:, 0:128], in_=ptw[:, 0:128],
                            pattern=[[1, 128]], compare_op=Alu.is_ge,
                            fill=0.0, base=0, channel_multiplier=-1)
                    return ptw

                def pv_row(kb, ptw):
                    if not pas:
                        pas.extend(ps_att.tile([128, 129], F32, tag="pa",
                                               name="pa")
                                   for _ in range(QC))
                    diag = kb >= q0
                    for qq in range(kb - q0 if diag else 0, QC):
                        off = (qq - (kb - q0)) if diag else qq
                        nc.tensor.matmul(
                            pas[qq], ptw[:, off * 128:(off + 1) * 128],
                            vA[(b, kb)],
                            start=(kb == 0), stop=(q0 + qq == kb))

                def tail(qq):
                    qb = q0 + qq
                    pa = pas[qq]
                    rv = work.tile([128, 1], F32, tag="rsum", name="rsum")
                    nc.vector.reciprocal(rv, pa[:, 128:129])
                    an = work.tile([128, 128], BF16, tag="attn_n",
                                   name="attn_n")
                    nc.vector.tensor_scalar_mul(an, pa[:, 0:128], rv)
                    tr = ps_tr.tile([128, 128], BF16, tag="tr", name="tr")
                    nc.tensor.transpose(tr, an, ident)
                    at = work.tile([128, 128], BF16, tag="attnT",
                                   name="attnT")
                    nc.vector.tensor_copy(at, tr)
                    nc.sync.dma_start(
                        out=a2a_in[(b, hl, qb // N_CORES)][qb % N_CORES],
                        in_=at)

                def fire():
                    if (q0 + QC) % N_CORES == 0:
                        be = (q0 + QC) // N_CORES - 1
                        nc.gpsimd.collective_compute(
                            "AllToAll", Alu.bypass,
                            replica_groups=[list(range(N_CORES))],
                            ins=[a2a_in[(b, hl, be)].opt()],
                            outs=[a2a_out[(b, hl, be)].opt()])

                rows = list(range(q0 + QC))
                prev = None
                for i, kb in enumerate(rows):
                    cur = score_row(kb)
                    if i == 0 and st["deferred"] is not None:
                        tl, fr = st["deferred"]
                        st["deferred"] = None
                        tl(QC - 1)
                        fr()
                    if prev is not None:
                        pv_row(rows[i - 1], prev)
                    if kb > q0:
                        tail(kb - q0 - 1)
                    prev = cur
                pv_row(rows[-1], prev)
                st["deferred"] = (tail, fire)

            def attn_finish(b, hl):
                tl, fr = attn_state[(b, hl)]["deferred"]
                attn_state[(b, hl)]["deferred"] = None
                tl(QC - 1)
                fr()

            # ---- out projection ------------------------------------------
            gs_t = {}

            def load_gs(b, blk):
                g = persist.tile([128, 16, 128], BF16, tag=f"aG_{b}_{blk}",
                                 name=f"aG_{b}_{blk}")
                gs_t[(b, blk)] = g
                gr = g.rearrange("p (r hl) t -> p r hl t", hl=2)
                for hl in range(2):
                    nc.sync.dma_start(
                        out=gr[:, :, hl, :],
                        in_=a2a_out[(b, hl, blk)][:, :, :]
                            .rearrange("r p t -> p r t"))

            def oproj_piece(b, blk, chunk):
                g = gs_t[(b, blk)]
                po = ps512.tile([128, 512], F32, tag="p512", name="p512")
                for idx in range(16):
                    nc.tensor.matmul(po, g[:, idx, :],
                                     ws_t[chunk][:, AD_OF[idx], :],
                                     start=(idx == 0), stop=(idx == 15))
                os_ = work.tile([128, 512], BF16, tag="os", name="os",
                                bufs=3)
                nc.scalar.activation(os_, po, Act.Copy)
                nc.scalar.dma_start(
                    out=out.ap()[b, blk, :, chunk * 512:(chunk + 1) * 512],
                    in_=os_)

            # ---- emission schedule ---------------------------------------
            def proj1_pair(p):
                for i in range(2):
                    tb = 2 * p + i
                    if tb + 2 < TB:
                        load_xt(1, tb + 2)
                    proj_block(1, tb)
                if p % 2 == 1:
                    fire_kv(1, p // 2)

            def dump(sbuf_tile, col=0):
                """debug: land something in `out` so the NEFF has output."""
                nc.scalar.dma_start(
                    out=out.ap()[0, 0, :, col * 128:(col + 1) * 128],
                    in_=sbuf_tile)

            if SCHEDULE == "seq":
                for tb in range(TB):
                    if tb + 2 < TB:
                        load_xt(0, tb + 2)
                    elif tb + 2 < TB + 3:
                        load_xt(1, tb + 2 - TB)
                    proj_block(0, tb)
                    if FIRE_KV and tb % 4 == 3:
                        fire_kv(0, tb // 4)
                if PREFIX == 1:
                    dump(qTbig[(0, 0)][:, 0:128])
                if PREFIX >= 2:
                    for p in range(8):
                        proj1_pair(p)
                    for g in range(8):
                        postag_group(0, g)
                    if PREFIX == 2:
                        dump(kTt[(0, 0)])
                if PREFIX >= 3:
                    for hl in range(2):
                        for j in range(4):
                            attn_chunk(0, hl, j)
                        attn_finish(0, hl)
                    for g in range(8):
                        postag_group(1, g)
                    if PREFIX == 3:
                        dump(kTt[(1, 0)])
                if PREFIX >= 4:
                    for c in range(4):
                        load_ws(c)
                    for hl in range(2):
                        for j in range(4):
                            attn_chunk(1, hl, j)
                        attn_finish(1, hl)
                    for b in range(B):
                        for blk in range(BPS):
                            load_gs(b, blk)
                            for c in range(4):
                                oproj_piece(b, blk, c)
            else:
                # proj(0) with per-4tb kv fires; post_ag(0) interleaved
                for tb in range(TB):
                    if tb + 2 < TB:
                        load_xt(0, tb + 2)
                    elif tb + 2 < TB + 3:
                        load_xt(1, tb + 2 - TB)
                    proj_block(0, tb)
                    if tb % 4 == 3:
                        fire_kv(0, tb // 4)
                    if tb == 9:
                        postag_group(0, 0)
                    if tb == 11:
                        postag_group(0, 1)
                    if tb == 13:
                        postag_group(0, 2)
                    if tb == 15:
                        postag_group(0, 3)
                postag_group(0, 4)
                postag_group(0, 5)
                attn_chunk(0, 0, 0)
                postag_group(0, 6)
                postag_group(0, 7)
                proj1_pair(0)
                attn_chunk(0, 0, 1)
                proj1_pair(1)
                attn_chunk(0, 0, 2)
                proj1_pair(2)
                attn_chunk(0, 0, 3)
                proj1_pair(3)
                attn_finish(0, 0)
                attn_chunk(0, 1, 0)
                proj1_pair(4)
                attn_chunk(0, 1, 1)
                proj1_pair(5)
                attn_chunk(0, 1, 2)
                proj1_pair(6)
                attn_chunk(0, 1, 3)
                proj1_pair(7)
                attn_finish(0, 1)
                load_ws(0)
                load_ws(1)
                postag_group(1, 0)
                postag_group(1, 1)
                attn_chunk(1, 0, 0)
                postag_group(1, 2)
                postag_group(1, 3)
                attn_chunk(1, 0, 1)
                postag_group(1, 4)
                postag_group(1, 5)
                load_ws(2)
                load_ws(3)
                load_gs(0, 0)
                oproj_piece(0, 0, 0)
                oproj_piece(0, 0, 1)
                attn_chunk(1, 0, 2)
                postag_group(1, 6)
                postag_group(1, 7)
                oproj_piece(0, 0, 2)
                oproj_piece(0, 0, 3)
                attn_chunk(1, 0, 3)
                attn_finish(1, 0)
                load_gs(0, 1)
                oproj_piece(0, 1, 0)
                oproj_piece(0, 1, 1)
                attn_chunk(1, 1, 0)
                oproj_piece(0, 1, 2)
                oproj_piece(0, 1, 3)
                attn_chunk(1, 1, 1)
                attn_chunk(1, 1, 2)
                load_gs(1, 0)
                oproj_piece(1, 0, 0)
                oproj_piece(1, 0, 1)
                attn_chunk(1, 1, 3)
                attn_finish(1, 1)
                oproj_piece(1, 0, 2)
                oproj_piece(1, 0, 3)
                load_gs(1, 1)
                for c in range(4):
                    oproj_piece(1, 1, c)
    nc.compile()
    return nc


# ---- host side ----------------------------------------------------------

def _yarn_tables(t_tokens):
    inv = 1.0 / ROPE_BASE ** (np.arange(0, D_HEAD, 2, dtype=np.float32) / D_HEAD)
    wavelengths = 2.0 * math.pi / inv
    low_wl = ORIG_MAX_LEN / BETA_SLOW
    high_wl = ORIG_MAX_LEN / BETA_FAST
    gamma = np.clip((low_wl - wavelengths) / (low_wl - high_wl), 0.0, 1.0)
    inv_freq = (gamma * inv + (1.0 - gamma) * inv / YARN_SCALE).astype(np.float32)
    t = np.arange(t_tokens, dtype=np.float32)
    freqs = np.outer(t, inv_freq)                      # (T, 64)
    emb = np.concatenate([freqs, freqs], axis=-1)      # (T, 128)
    return np.cos(emb).astype(np.float32), np.sin(emb).astype(np.float32)


def _host_prep(x, Wq, Wkv, Wo, q_norm_w, k_norm_w):
    bf = ml_dtypes.bfloat16
    xT = np.ascontiguousarray(x.transpose(0, 2, 1)).astype(bf)   # (B, D, T)
    cos, sin = _yarn_tables(T)
    sinF = sin.copy()
    sinF[:, :64] *= -1.0
    # rms weight applies to x before rope; the sin term reads the *rotated*
    # input, so its weight index is the input position (rolled by 64).
    wq_roll = np.concatenate([q_norm_w[64:], q_norm_w[:64]])
    wk_roll = np.concatenate([k_norm_w[64:], k_norm_w[:64]])
    cq = (cos * q_norm_w[None, :] * ATTN_SCALE).astype(bf)
    sq = (sinF * wq_roll[None, :] * ATTN_SCALE).astype(bf)
    c2q = np.ascontiguousarray(np.concatenate([cq, cq], axis=1))  # (T, 256)
    s2q = np.ascontiguousarray(np.concatenate([sq, sq], axis=1))
    ckt = np.ascontiguousarray((cos * k_norm_w[None, :]).astype(bf))
    skt = np.ascontiguousarray((sinF * wk_roll[None, :]).astype(bf))
    Wk, Wv = Wkv[:, :N_KV * D_HEAD], Wkv[:, N_KV * D_HEAD:]
    wo_t = np.ascontiguousarray(Wo.astype(bf).reshape(KT, 128, D_MODEL))
    in_maps = []
    for c in range(N_CORES):
        g, (ha, hb) = _core_heads(c)
        kv_half = (Wk if c % 2 == 0 else Wv)[:, g * 128:(g + 1) * 128]
        wcols = np.concatenate([
            Wq[:, ha * 128:(ha + 1) * 128], Wq[:, hb * 128:(hb + 1) * 128],
            kv_half,
        ], axis=1).astype(bf)                               # (D, 384)
        in_maps.append({
            "xT": xT, "wc": np.ascontiguousarray(wcols.reshape(KT, 128, 384)),
            "wo": wo_t,
            "c2q": c2q, "s2q": s2q, "ck": ckt, "sk": skt,
        })
    return in_maps


def _assemble(results):
    out = np.empty((B, T, D_MODEL), dtype=np.float32)
    for c in range(N_CORES):
        oc = results[c]["out"]              # (B, BPS, 128, D) bf16
        for b in range(B):
            for blk in range(BPS):
                t0 = (c + blk * N_CORES) * 128
                out[b, t0:t0 + 128, :] = oc[b, blk].astype(np.float32)
    return out


_NC_CACHE = {}


def kernel(x, Wq, Wkv, Wo, q_norm_w, k_norm_w):
    x = np.asarray(x, dtype=np.float32)
    Wq = np.asarray(Wq, dtype=np.float32)
    Wkv = np.asarray(Wkv, dtype=np.float32)
    Wo = np.asarray(Wo, dtype=np.float32)
    q_norm_w = np.asarray(q_norm_w, dtype=np.float32)
    k_norm_w = np.asarray(k_norm_w, dtype=np.float32)

    if "nc" not in _NC_CACHE:
        _NC_CACHE["nc"] = build_nc()
    nc = _NC_CACHE["nc"]
    in_maps = _host_prep(x, Wq, Wkv, Wo, q_norm_w, k_norm_w)
    res = run_bass_kernel_spmd(nc, in_maps, core_ids=list(range(N_CORES)))
    return _assemble(res.results)


if __name__ == "__main__":
    rng = np.random.default_rng(0)
    x = rng.standard_normal((B, T, D_MODEL), dtype=np.float32)
    Wq = rng.standard_normal((D_MODEL, N_Q * D_HEAD), dtype=np.float32) * 0.02
    Wkv = rng.standard_normal((D_MODEL, 2 * N_KV * D_HEAD), dtype=np.float32) * 0.02
    Wo = rng.standard_normal((N_Q * D_HEAD, D_MODEL), dtype=np.float32) * 0.02
    w1 = np.ones(D_HEAD, dtype=np.float32)
    o = kernel(x, Wq, Wkv, Wo, w1, w1)
    print(o.shape, o.dtype, float(np.abs(o).mean()))
